# revision 22
# baseline (speedup 1.0000x reference)
"""Trainium2 Bass kernel for GNN edge attention (nn_Attention_16338055594502).

For each edge e with endpoints (row[e], col[e]):
    logits[e] = x[row[e]] @ W[:C] + x[col[e]] @ W[C:] + b        # [E, H]
    alpha[e]  = sigmoid(logits[e]) * edge_attr[e]
    alpha[e]  = 1.0 where row[e] == col[e]
Returns (alpha, edge_index).

Strategy (8 NeuronCores, edge-parallel; x/W/b replicated):
  - On device, precompute per-node projection tables
        tabr[n] = x[n] @ W[:C] + b     tabc[n] = x[n] @ W[C:]    # [N, 8] f32
    so per-edge work reduces to two 32-byte row gathers.
  - Tables are stored with 256-byte row stride (DMAGatherAnt requirement).
  - Per core, two 80000-index dma_gather (ant SWDGE bulk gather)
    instructions fetch tabr[row[e]] / tabc[col[e]] into SBUF, landing edge
    slot i at dst[i % 128, i // 128, :].
  - DVE/ACT: add, sigmoid, * edge_attr, and alpha = max(alpha, row==col)
    (exact: sigmoid(.)*ea < 1 for ea in [0,1)).
  - Host stages indices in the ant wrapped int16 layout and permutes the
    per-core output rows back to edge order (pure data movement).
"""

import sys

sys.path.insert(0, "/opt/trn_rl_repo")

import numpy as np

from concourse import bacc, bass, mybir
from concourse import ap_utils
import concourse.tile as tile
from concourse.bass_utils import run_bass_kernel_spmd
from concourse.masks import make_identity

N, C, E, H = 10000, 128, 640000, 8
P = 128
N_CORES = 8
EC = E // N_CORES              # 80000 edges per core
ET = 5                         # edge tiles per core (dma_gather <= ~16K idxs)
NIT = EC // ET                 # 16000 indices per gather
JT = NIT // P                  # 125 gather columns per tile
J = EC // P                    # 625 gather columns total
W16T = NIT // 16               # 1000 wrapped-idx columns per tile
NPAD = ((N + P - 1) // P) * P  # 10112
NT = NPAD // P                 # 79 node tiles
TSTRIDE = 64                   # table row stride in f32 (256 bytes)

FP32 = mybir.dt.float32
I16 = mybir.dt.int16


def dma_gather_raw(eng, out_ap, in_ap, idxs_ap, num_idxs, elem_size,
                   elem_step, queue_num=0):
    """bass.BassGpSimd.dma_gather (HBM source, transpose=False) minus the
    elem_size_bytes % 256 assert: payload may be smaller than the 256B-
    aligned row stride."""
    assert idxs_ap.dtype == mybir.dt.int16
    assert in_ap.dtype == out_ap.dtype
    assert in_ap.space == bass.MemorySpace.DRAM
    assert idxs_ap.space == bass.MemorySpace.SBUF
    assert out_ap.space == bass.MemorySpace.SBUF
    assert ap_utils.ap_is_contiguous(out_ap.ap[1:])
    assert ap_utils.ap_is_contiguous(idxs_ap.ap[1:])
    assert in_ap.ap[-1][1] == out_ap.ap[-1][1] == elem_size
    assert out_ap.ap[0][1] * out_ap.ap[1][1] == num_idxs
    assert in_ap.ap[0][0] == elem_step
    stride_bytes = elem_step * mybir.dt.size(in_ap.dtype)
    stride_bytes_256 = stride_bytes // 256
    assert stride_bytes_256 * 256 == stride_bytes and stride_bytes_256 < 256

    _in_ap = eng.lower_ap_dma(in_ap, for_custom_bir_dma=True)
    _idxs_ap = eng.lower_ap(idxs_ap)
    _out_ap = eng.lower_ap(out_ap)
    return eng.add_instruction(
        mybir.InstDMAGatherAnt(
            name=eng.bass.get_next_instruction_name(),
            ins=[*_in_ap, _idxs_ap, eng.lower_val_access(eng.to_reg(num_idxs))],
            outs=[_out_ap],
            transpose=False,
            num_idxs=num_idxs,
            elem_size=elem_size,
            stride_bytes_256=stride_bytes_256,
            gen_mode=0,
            single_packet=False,
            queue_num=queue_num,
            sbuf_tokens_per_rank=0,
            sbuf_free_dim_per_rank=0,
            sbuf_free_dim_pad_per_rank=0,
            sbuf_byte_offset=0,
        )
    )


NQ = 4  # SWDGE queues; gathers round-robin across them


def build_nc(repeat=1):
    nc = bacc.Bacc(num_swdge_queues=NQ)

    xp = nc.declare_dram_parameter("xp", [NPAD, C], FP32, isOutput=False)
    wrc = nc.declare_dram_parameter("wrc", [C, 2 * H], FP32, isOutput=False)
    brep = nc.declare_dram_parameter("brep", [P, 2 * H], FP32, isOutput=False)
    roww = nc.declare_dram_parameter("roww", [ET * P, W16T], I16, isOutput=False)
    colw = nc.declare_dram_parameter("colw", [ET * P, W16T], I16, isOutput=False)
    rowd = nc.declare_dram_parameter("rowd", [P, J], FP32, isOutput=False)
    cold = nc.declare_dram_parameter("cold", [P, J], FP32, isOutput=False)
    ead = nc.declare_dram_parameter("ead", [P, J], FP32, isOutput=False)
    out = nc.declare_dram_parameter("out", [EC, H], FP32, isOutput=True)

    with tile.TileContext(nc) as tc:
        with (
            tc.tile_pool(name="const", bufs=1) as const_pool,
            tc.tile_pool(name="xload", bufs=3) as xpool,
            tc.tile_pool(name="xt", bufs=3) as xtpool,
            tc.tile_pool(name="tabsb", bufs=3) as tabpool,
            tc.tile_pool(name="psum_t", bufs=2, space="PSUM") as psum_t,
            tc.tile_pool(name="psum_m", bufs=2, space="PSUM") as psum_m,
            tc.tile_pool(name="idx", bufs=3) as ipool,
            tc.tile_pool(name="aux", bufs=3) as apool,
            tc.tile_pool(name="gath", bufs=3) as gpool,
            tc.tile_pool(name="dram", bufs=1, space="DRAM") as dram_pool,
        ):
            identity = const_pool.tile([P, P], FP32)
            make_identity(nc, identity[:])
            wrc_sb = const_pool.tile([C, 2 * H], FP32)
            nc.sync.dma_start(out=wrc_sb[:], in_=wrc[:, :])
            brep_sb = const_pool.tile([P, 2 * H], FP32)
            nc.sync.dma_start(out=brep_sb[:], in_=brep[:, :])

            tabr = dram_pool.tile([NPAD, TSTRIDE], FP32)
            tabc = dram_pool.tile([NPAD, TSTRIDE], FP32)

            # ---- per-node tables: tabr[n,:8] = x@Wr + b ; tabc[n,:8] = x@Wc
            for i in range(NT):
                xt = xpool.tile([P, C], FP32)
                nc.sync.dma_start(out=xt[:], in_=xp[i * P:(i + 1) * P, :])
                xT_ps = psum_t.tile([P, P], FP32, space="PSUM")
                nc.tensor.transpose(out=xT_ps[:], in_=xt[:], identity=identity[:])
                xT_sb = xtpool.tile([P, P], FP32)
                nc.vector.tensor_copy(out=xT_sb[:], in_=xT_ps[:])
                tps = psum_m.tile([P, 2 * H], FP32, space="PSUM")
                nc.tensor.matmul(
                    out=tps[:], lhsT=xT_sb[:], rhs=wrc_sb[:], start=True, stop=True
                )
                tsb = tabpool.tile([P, 2 * H], FP32)
                nc.vector.tensor_add(out=tsb[:], in0=tps[:], in1=brep_sb[:])
                nc.sync.dma_start(out=tabr[i * P:(i + 1) * P, 0:H], in_=tsb[:, 0:H])
                nc.sync.dma_start(
                    out=tabc[i * P:(i + 1) * P, 0:H], in_=tsb[:, H:2 * H]
                )

            # ---- per-tile: two 16000-idx gathers + elementwise tail
            out_v = out[:, :].rearrange("(p t j) h -> p t (j h)", p=P, t=ET)
            for t in [t for _ in range(repeat) for t in range(ET)]:
                roww_sb = ipool.tile([P, W16T], I16, tag="roww")
                nc.sync.dma_start(
                    out=roww_sb[:], in_=roww[t * P:(t + 1) * P, :]
                )
                colw_sb = ipool.tile([P, W16T], I16, tag="colw")
                nc.sync.dma_start(
                    out=colw_sb[:], in_=colw[t * P:(t + 1) * P, :]
                )

                # edge slot i (of this tile) -> dst[i % 128, i // 128, :]
                gr = gpool.tile([P, JT * H], FP32, tag="gr")
                dma_gather_raw(
                    nc.gpsimd,
                    gr[:].rearrange("p (j e) -> p j e", e=H),
                    tabr[:, 0:H],
                    roww_sb[:],
                    num_idxs=NIT,
                    elem_size=H,
                    elem_step=TSTRIDE,
                    queue_num=(2 * t) % NQ,
                )
                gc = gpool.tile([P, JT * H], FP32, tag="gc")
                dma_gather_raw(
                    nc.gpsimd,
                    gc[:].rearrange("p (j e) -> p j e", e=H),
                    tabc[:, 0:H],
                    colw_sb[:],
                    num_idxs=NIT,
                    elem_size=H,
                    elem_step=TSTRIDE,
                    queue_num=(2 * t + 1) % NQ,
                )

                ead_sb = apool.tile([P, JT], FP32, tag="ead")
                nc.sync.dma_start(
                    out=ead_sb[:], in_=ead[:, t * JT:(t + 1) * JT]
                )
                rowd_sb = apool.tile([P, JT], FP32, tag="rowd")
                nc.sync.dma_start(
                    out=rowd_sb[:], in_=rowd[:, t * JT:(t + 1) * JT]
                )
                cold_sb = apool.tile([P, JT], FP32, tag="cold")
                nc.sync.dma_start(
                    out=cold_sb[:], in_=cold[:, t * JT:(t + 1) * JT]
                )

                nc.vector.tensor_add(out=gr[:], in0=gr[:], in1=gc[:])
                nc.scalar.activation(
                    out=gr[:], in_=gr[:],
                    func=mybir.ActivationFunctionType.Sigmoid,
                )
                gr3 = gr[:].rearrange("p (j e) -> p j e", e=H)
                ea3 = (
                    ead_sb[:]
                    .rearrange("p (j o) -> p j o", o=1)
                    .to_broadcast([P, JT, H])
                )
                nc.vector.tensor_mul(out=gr3, in0=gr3, in1=ea3)

                mask = apool.tile([P, JT], FP32, tag="mask")
                nc.vector.tensor_tensor(
                    out=mask[:], in0=rowd_sb[:], in1=cold_sb[:],
                    op=mybir.AluOpType.is_equal,
                )
                m3 = (
                    mask[:]
                    .rearrange("p (j o) -> p j o", o=1)
                    .to_broadcast([P, JT, H])
                )
                nc.vector.tensor_tensor(
                    out=gr3, in0=gr3, in1=m3, op=mybir.AluOpType.max
                )

                # out row (p*J + t*JT + j) holds edge slot t*NIT + j*128 + p
                nc.sync.dma_start(out=out_v[:, t], in_=gr[:])

    nc.compile()
    return nc


_NC = {}


def _get_nc(repeat=1):
    if repeat not in _NC:
        _NC[repeat] = build_nc(repeat)
    return _NC[repeat]


def _wrap16(a):
    """flat [EC] -> per-tile ant wrapped idx layout [ET*128, NIT//16]
    (tile t, idx i at [t*128 + i % 16 (+16g replicas), i // 16])."""
    w = a.reshape(ET, W16T, 16)
    out = np.empty((ET, P, W16T), a.dtype)
    for t in range(ET):
        out[t] = np.tile(w[t].T, (8, 1))
    return out.reshape(ET * P, W16T).copy()


def _stage_inputs(x, edge_index, edge_attr, W, b):
    xp = np.zeros((NPAD, C), np.float32)
    xp[:N] = np.asarray(x, np.float32)
    W = np.asarray(W, np.float32)
    wrc = np.concatenate([W[:C], W[C:]], axis=1).astype(np.float32)
    brep = np.zeros((P, 2 * H), np.float32)
    brep[:, :H] = np.asarray(b, np.float32)
    row = np.ascontiguousarray(edge_index[0]).astype(np.int64)
    col = np.ascontiguousarray(edge_index[1]).astype(np.int64)
    ea = np.ascontiguousarray(np.asarray(edge_attr, np.float32))

    in_maps = []
    for i in range(N_CORES):
        s = slice(i * EC, (i + 1) * EC)
        r, c, e = row[s], col[s], ea[s]
        # dst layout: tile t slot i -> [i % 128, t*JT + i // 128]
        rd = r.reshape(ET, JT, P).transpose(2, 0, 1).reshape(P, J).astype(np.float32)
        cd = c.reshape(ET, JT, P).transpose(2, 0, 1).reshape(P, J).astype(np.float32)
        ed = e.reshape(ET, JT, P).transpose(2, 0, 1).reshape(P, J).astype(np.float32)
        in_maps.append(
            dict(
                xp=xp,
                wrc=wrc,
                brep=brep,
                roww=_wrap16(r.astype(np.int16)),
                colw=_wrap16(c.astype(np.int16)),
                rowd=np.ascontiguousarray(rd),
                cold=np.ascontiguousarray(cd),
                ead=np.ascontiguousarray(ed),
            )
        )
    return in_maps


def run(inputs, trace=False, repeat=1, **kw):
    nc = _get_nc(repeat)
    in_maps = _stage_inputs(**inputs)
    res = run_bass_kernel_spmd(
        nc, in_maps, core_ids=list(range(N_CORES)), trace=trace, **kw
    )
    # out row (p*J + t*JT + j) holds edge slot t*NIT + j*128 + p
    perm = (
        np.arange(EC).reshape(ET, JT, P).transpose(2, 0, 1).reshape(EC)
    )  # perm[p*J + t*JT + j] = t*NIT + j*128 + p
    alpha = np.empty((E, H), np.float32)
    for i in range(N_CORES):
        shard = np.asarray(res.results[i]["out"])
        alpha[i * EC + perm] = shard
    return alpha, res


def kernel(x, edge_index, edge_attr, W, b):
    alpha, _ = run(dict(x=x, edge_index=edge_index, edge_attr=edge_attr, W=W, b=b))
    return alpha, edge_index


# revision 31
# speedup vs baseline: 1.3072x; 1.3072x over previous
"""Trainium2 Bass kernel for GNN edge attention (nn_Attention_16338055594502).

For each edge e with endpoints (row[e], col[e]):
    logits[e] = x[row[e]] @ W[:C] + x[col[e]] @ W[C:] + b        # [E, H]
    alpha[e]  = sigmoid(logits[e]) * edge_attr[e]
    alpha[e]  = 1.0 where row[e] == col[e]
Returns (alpha, edge_index).

Strategy (8 NeuronCores, edge-parallel; x/W/b replicated):
  - On device, precompute per-node projection tables
        tabr[n] = x[n] @ W[:C] + b     tabc[n] = x[n] @ W[C:]    # [N, 8] f32
    so per-edge work reduces to two 32-byte row gathers.
  - Tables are stored with 256-byte row stride (DMAGatherAnt requirement).
  - Per core, 5 tile-pairs of 16000-index dma_gather (ant SWDGE bulk
    gather; ~16K idxs is the per-instruction limit) fetch tabr[row[e]] /
    tabc[col[e]] into SBUF, landing edge slot i at dst[i%128, i//128, :].
    The 10 gathers round-robin over 4 SWDGE queues (num_swdge_queues=4) —
    measured fastest config: fewer/bigger gathers beat balanced smaller
    ones, and sorted (duplicate-run) index streams are slower than random.
  - DVE/ACT: add, sigmoid, * edge_attr, and alpha = max(alpha, row==col)
    (exact: sigmoid(.)*ea < 1 for ea in [0,1)).
  - Host stages indices in the ant wrapped int16 layout and permutes the
    per-core output rows back to edge order (pure data movement).
"""

import sys

sys.path.insert(0, "/opt/trn_rl_repo")

import numpy as np

from concourse import bacc, bass, mybir
from concourse import ap_utils
import concourse.tile as tile
from concourse.bass_utils import run_bass_kernel_spmd
from concourse.masks import make_identity

N, C, E, H = 10000, 128, 640000, 8
P = 128
N_CORES = 8
EC = E // N_CORES              # 80000 edges per core
ET = 5                         # edge tiles per core (dma_gather <= ~16K idxs)
NIT = EC // ET                 # 16000 indices per gather
JT = NIT // P                  # 125 gather columns per tile
J = EC // P                    # 625 gather columns total
W16T = NIT // 16               # 1000 wrapped-idx columns per tile
NPAD = ((N + P - 1) // P) * P  # 10112
NT = NPAD // P                 # 79 node tiles
TSTRIDE = 64                   # table row stride in f32 (256 bytes)

FP32 = mybir.dt.float32
I16 = mybir.dt.int16


def dma_gather_raw(eng, out_ap, in_ap, idxs_ap, num_idxs, elem_size,
                   elem_step, queue_num=0):
    """bass.BassGpSimd.dma_gather (HBM source, transpose=False) minus the
    elem_size_bytes % 256 assert: payload may be smaller than the 256B-
    aligned row stride."""
    assert idxs_ap.dtype == mybir.dt.int16
    assert in_ap.dtype == out_ap.dtype
    assert in_ap.space == bass.MemorySpace.DRAM
    assert idxs_ap.space == bass.MemorySpace.SBUF
    assert out_ap.space == bass.MemorySpace.SBUF
    assert ap_utils.ap_is_contiguous(out_ap.ap[1:])
    assert ap_utils.ap_is_contiguous(idxs_ap.ap[1:])
    assert in_ap.ap[-1][1] == out_ap.ap[-1][1] == elem_size
    assert out_ap.ap[0][1] * out_ap.ap[1][1] == num_idxs
    assert in_ap.ap[0][0] == elem_step
    stride_bytes = elem_step * mybir.dt.size(in_ap.dtype)
    stride_bytes_256 = stride_bytes // 256
    assert stride_bytes_256 * 256 == stride_bytes and stride_bytes_256 < 256

    _in_ap = eng.lower_ap_dma(in_ap, for_custom_bir_dma=True)
    _idxs_ap = eng.lower_ap(idxs_ap)
    _out_ap = eng.lower_ap(out_ap)
    return eng.add_instruction(
        mybir.InstDMAGatherAnt(
            name=eng.bass.get_next_instruction_name(),
            ins=[*_in_ap, _idxs_ap, eng.lower_val_access(eng.to_reg(num_idxs))],
            outs=[_out_ap],
            transpose=False,
            num_idxs=num_idxs,
            elem_size=elem_size,
            stride_bytes_256=stride_bytes_256,
            gen_mode=0,
            single_packet=False,
            queue_num=queue_num,
            sbuf_tokens_per_rank=0,
            sbuf_free_dim_per_rank=0,
            sbuf_free_dim_pad_per_rank=0,
            sbuf_byte_offset=0,
        )
    )


NQ = 4  # SWDGE queues; gathers round-robin across them


def build_nc(repeat=1):
    nc = bacc.Bacc(num_swdge_queues=NQ)

    xp = nc.declare_dram_parameter("xp", [NPAD, C], FP32, isOutput=False)
    wrc = nc.declare_dram_parameter("wrc", [C, 2 * H], FP32, isOutput=False)
    brep = nc.declare_dram_parameter("brep", [P, 2 * H], FP32, isOutput=False)
    roww = nc.declare_dram_parameter("roww", [ET * P, W16T], I16, isOutput=False)
    colw = nc.declare_dram_parameter("colw", [ET * P, W16T], I16, isOutput=False)
    rowd = nc.declare_dram_parameter("rowd", [P, J], FP32, isOutput=False)
    cold = nc.declare_dram_parameter("cold", [P, J], FP32, isOutput=False)
    ead = nc.declare_dram_parameter("ead", [P, J], FP32, isOutput=False)
    out = nc.declare_dram_parameter("out", [EC, H], FP32, isOutput=True)

    with tile.TileContext(nc) as tc:
        with (
            tc.tile_pool(name="const", bufs=1) as const_pool,
            tc.tile_pool(name="xload", bufs=3) as xpool,
            tc.tile_pool(name="xt", bufs=3) as xtpool,
            tc.tile_pool(name="tabsb", bufs=3) as tabpool,
            tc.tile_pool(name="psum_t", bufs=2, space="PSUM") as psum_t,
            tc.tile_pool(name="psum_m", bufs=2, space="PSUM") as psum_m,
            tc.tile_pool(name="idx", bufs=3) as ipool,
            tc.tile_pool(name="aux", bufs=3) as apool,
            tc.tile_pool(name="gath", bufs=3) as gpool,
            tc.tile_pool(name="dram", bufs=1, space="DRAM") as dram_pool,
        ):
            identity = const_pool.tile([P, P], FP32)
            make_identity(nc, identity[:])
            wrc_sb = const_pool.tile([C, 2 * H], FP32)
            nc.sync.dma_start(out=wrc_sb[:], in_=wrc[:, :])
            brep_sb = const_pool.tile([P, 2 * H], FP32)
            nc.sync.dma_start(out=brep_sb[:], in_=brep[:, :])

            tabr = dram_pool.tile([NPAD, TSTRIDE], FP32)
            tabc = dram_pool.tile([NPAD, TSTRIDE], FP32)

            # ---- per-node tables: tabr[n,:8] = x@Wr + b ; tabc[n,:8] = x@Wc
            for i in range(NT):
                xt = xpool.tile([P, C], FP32)
                nc.sync.dma_start(out=xt[:], in_=xp[i * P:(i + 1) * P, :])
                xT_ps = psum_t.tile([P, P], FP32, space="PSUM")
                nc.tensor.transpose(out=xT_ps[:], in_=xt[:], identity=identity[:])
                xT_sb = xtpool.tile([P, P], FP32)
                nc.vector.tensor_copy(out=xT_sb[:], in_=xT_ps[:])
                tps = psum_m.tile([P, 2 * H], FP32, space="PSUM")
                nc.tensor.matmul(
                    out=tps[:], lhsT=xT_sb[:], rhs=wrc_sb[:], start=True, stop=True
                )
                tsb = tabpool.tile([P, 2 * H], FP32)
                nc.vector.tensor_add(out=tsb[:], in0=tps[:], in1=brep_sb[:])
                nc.sync.dma_start(out=tabr[i * P:(i + 1) * P, 0:H], in_=tsb[:, 0:H])
                nc.sync.dma_start(
                    out=tabc[i * P:(i + 1) * P, 0:H], in_=tsb[:, H:2 * H]
                )

            # ---- per-tile: two 16000-idx gathers + elementwise tail
            out_v = out[:, :].rearrange("(p t j) h -> p t (j h)", p=P, t=ET)
            for t in [t for _ in range(repeat) for t in range(ET)]:
                roww_sb = ipool.tile([P, W16T], I16, tag="roww")
                nc.sync.dma_start(
                    out=roww_sb[:], in_=roww[t * P:(t + 1) * P, :]
                )
                colw_sb = ipool.tile([P, W16T], I16, tag="colw")
                nc.sync.dma_start(
                    out=colw_sb[:], in_=colw[t * P:(t + 1) * P, :]
                )

                # edge slot i (of this tile) -> dst[i % 128, i // 128, :]
                gr = gpool.tile([P, JT * H], FP32, tag="gr")
                dma_gather_raw(
                    nc.gpsimd,
                    gr[:].rearrange("p (j e) -> p j e", e=H),
                    tabr[:, 0:H],
                    roww_sb[:],
                    num_idxs=NIT,
                    elem_size=H,
                    elem_step=TSTRIDE,
                    queue_num=(2 * t) % NQ,
                )
                gc = gpool.tile([P, JT * H], FP32, tag="gc")
                dma_gather_raw(
                    nc.gpsimd,
                    gc[:].rearrange("p (j e) -> p j e", e=H),
                    tabc[:, 0:H],
                    colw_sb[:],
                    num_idxs=NIT,
                    elem_size=H,
                    elem_step=TSTRIDE,
                    queue_num=(2 * t + 1) % NQ,
                )

                ead_sb = apool.tile([P, JT], FP32, tag="ead")
                nc.sync.dma_start(
                    out=ead_sb[:], in_=ead[:, t * JT:(t + 1) * JT]
                )
                rowd_sb = apool.tile([P, JT], FP32, tag="rowd")
                nc.sync.dma_start(
                    out=rowd_sb[:], in_=rowd[:, t * JT:(t + 1) * JT]
                )
                cold_sb = apool.tile([P, JT], FP32, tag="cold")
                nc.sync.dma_start(
                    out=cold_sb[:], in_=cold[:, t * JT:(t + 1) * JT]
                )

                nc.vector.tensor_add(out=gr[:], in0=gr[:], in1=gc[:])
                nc.scalar.activation(
                    out=gr[:], in_=gr[:],
                    func=mybir.ActivationFunctionType.Sigmoid,
                )
                gr3 = gr[:].rearrange("p (j e) -> p j e", e=H)
                ea3 = (
                    ead_sb[:]
                    .rearrange("p (j o) -> p j o", o=1)
                    .to_broadcast([P, JT, H])
                )
                nc.vector.tensor_mul(out=gr3, in0=gr3, in1=ea3)

                mask = apool.tile([P, JT], FP32, tag="mask")
                nc.vector.tensor_tensor(
                    out=mask[:], in0=rowd_sb[:], in1=cold_sb[:],
                    op=mybir.AluOpType.is_equal,
                )
                m3 = (
                    mask[:]
                    .rearrange("p (j o) -> p j o", o=1)
                    .to_broadcast([P, JT, H])
                )
                nc.vector.tensor_tensor(
                    out=gr3, in0=gr3, in1=m3, op=mybir.AluOpType.max
                )

                # out row (p*J + t*JT + j) holds edge slot t*NIT + j*128 + p
                nc.sync.dma_start(out=out_v[:, t], in_=gr[:])

    nc.compile()
    return nc


_NC = {}


def _get_nc(repeat=1):
    if repeat not in _NC:
        _NC[repeat] = build_nc(repeat)
    return _NC[repeat]


def _wrap16(a):
    """flat [EC] -> per-tile ant wrapped idx layout [ET*128, NIT//16]
    (tile t, idx i at [t*128 + i % 16 (+16g replicas), i // 16])."""
    w = a.reshape(ET, W16T, 16)
    out = np.empty((ET, P, W16T), a.dtype)
    for t in range(ET):
        out[t] = np.tile(w[t].T, (8, 1))
    return out.reshape(ET * P, W16T).copy()


def _stage_inputs(x, edge_index, edge_attr, W, b):
    xp = np.zeros((NPAD, C), np.float32)
    xp[:N] = np.asarray(x, np.float32)
    W = np.asarray(W, np.float32)
    wrc = np.concatenate([W[:C], W[C:]], axis=1).astype(np.float32)
    brep = np.zeros((P, 2 * H), np.float32)
    brep[:, :H] = np.asarray(b, np.float32)
    row = np.ascontiguousarray(edge_index[0]).astype(np.int64)
    col = np.ascontiguousarray(edge_index[1]).astype(np.int64)
    ea = np.ascontiguousarray(np.asarray(edge_attr, np.float32))

    in_maps = []
    for i in range(N_CORES):
        s = slice(i * EC, (i + 1) * EC)
        r, c, e = row[s], col[s], ea[s]
        # dst layout: tile t slot i -> [i % 128, t*JT + i // 128]
        rd = r.reshape(ET, JT, P).transpose(2, 0, 1).reshape(P, J).astype(np.float32)
        cd = c.reshape(ET, JT, P).transpose(2, 0, 1).reshape(P, J).astype(np.float32)
        ed = e.reshape(ET, JT, P).transpose(2, 0, 1).reshape(P, J).astype(np.float32)
        in_maps.append(
            dict(
                xp=xp,
                wrc=wrc,
                brep=brep,
                roww=_wrap16(r.astype(np.int16)),
                colw=_wrap16(c.astype(np.int16)),
                rowd=np.ascontiguousarray(rd),
                cold=np.ascontiguousarray(cd),
                ead=np.ascontiguousarray(ed),
            )
        )
    return in_maps


def run(inputs, trace=False, repeat=1, **kw):
    nc = _get_nc(repeat)
    in_maps = _stage_inputs(**inputs)
    res = run_bass_kernel_spmd(
        nc, in_maps, core_ids=list(range(N_CORES)), trace=trace, **kw
    )
    # out row (p*J + t*JT + j) holds edge slot t*NIT + j*128 + p
    perm = (
        np.arange(EC).reshape(ET, JT, P).transpose(2, 0, 1).reshape(EC)
    )  # perm[p*J + t*JT + j] = t*NIT + j*128 + p
    alpha = np.empty((E, H), np.float32)
    for i in range(N_CORES):
        shard = np.asarray(res.results[i]["out"])
        alpha[i * EC + perm] = shard
    return alpha, res


def kernel(x, edge_index, edge_attr, W, b):
    alpha, _ = run(dict(x=x, edge_index=edge_index, edge_attr=edge_attr, W=W, b=b))
    return alpha, edge_index
